# revision 5
# baseline (speedup 1.0000x reference)
"""Trainium2 Bass kernel for per-expert MoE FFN (gate/up/silu/down).

Problem shapes (hardcoded):
  expert_tokens        [2048, 2048] f32   (= E*T tokens, H hidden; sorted by expert)
  expert_tokens_count  [32] int64         (constant 64 per expert; unused)
  gate_proj            [32, 2048, 768] f32
  up_proj              [32, 2048, 768] f32
  down_proj            [32, 768, 2048] f32
  out                  [2048, 2048] f32

Sharding: expert-parallel across 8 NeuronCores - core c owns experts
[4c, 4c+4) and their token chunks (rows [256c, 256c+256)).  The
"all-to-all" of the hint is trivial here because tokens arrive already
sorted by expert, so the shard/gather happens host-side with numpy
slicing; each core computes its own tokens' outputs end to end.

Per-core dataflow (4 experts, T=64 tokens each):
  - x^T for all 4 experts is pre-transposed on host and loaded once to
    SBUF ([128, 16, 256] f32r view).  It is the matmul stationary
    operand (lhsT), so tokens-stationary / weights-moving keeps the
    TensorE streaming dimension large (N=384/512 >= 256, which is the
    condition for full-rate float32r matmuls).
  - gate/up:  g^ = x @ Wg, u = x @ Wu accumulated over 16 K-tiles into
    4 PSUM banks ([64, 384] x2 per matrix).
  - h = silu(g) * u  (ScalarE silu from PSUM, VectorE multiply).
  - h^T via 6 TensorE transposes (PSUM), then down: y = h @ Wd
    accumulated over 6 K-tiles into [64, 512] PSUM chunks.
  - y copied to an SBUF pair-tile ([128, 2048]) and DMA'd out once per
    expert pair for full-partition DMA efficiency.

Weights stream through double-buffered SBUF pools (786KB-1.5MB DMA
chunks); the kernel is HBM-DMA bound (~76MB of weights per core), so
everything else overlaps behind the weight stream.

float32r: hardware-rounded fp32 matmul mode (~1.6e-4 max rel err
measured on HW vs fp64, vs 4x slower exact fp32).
"""

import functools

import numpy as np

N_CORES = 8
E = 32                      # total experts
E_PER_CORE = E // N_CORES   # 4
T = 64                      # tokens per expert
H = 2048                    # hidden
F = 768                     # intermediate
KH = H // 128               # 16 K-tiles for gate/up
KF = F // 128               # 6 K-tiles for down
TC = E_PER_CORE * T         # 256 tokens per core


@functools.lru_cache(maxsize=1)
def _build_nc():
    from concourse import bacc
    import concourse.mybir as mybir
    import concourse.tile as tile
    from concourse.masks import make_identity

    f32 = mybir.dt.float32
    f32r = mybir.dt.float32r

    nc = bacc.Bacc(
        "TRN2", target_bir_lowering=False, debug=False, num_devices=N_CORES
    )
    xT = nc.declare_dram_parameter("xT", [H, TC], f32r, isOutput=False)
    wg = nc.declare_dram_parameter("wg", [E_PER_CORE, H, F], f32r, isOutput=False)
    wu = nc.declare_dram_parameter("wu", [E_PER_CORE, H, F], f32r, isOutput=False)
    wd = nc.declare_dram_parameter("wd", [E_PER_CORE, F, H], f32r, isOutput=False)
    out = nc.declare_dram_parameter("out", [TC, H], f32, isOutput=True)

    FH = F // 2  # 384, gate/up PSUM chunk width
    NH = 512     # down-proj PSUM chunk width
    NHC = H // NH  # 4 chunks

    with tile.TileContext(nc) as tc:
        with (
            tc.tile_pool(name="const", bufs=1) as constp,
            tc.tile_pool(name="xt", bufs=1) as xtp,
            tc.tile_pool(name="wgp", bufs=6) as wgp,
            tc.tile_pool(name="wup", bufs=6) as wup,
            tc.tile_pool(name="wdp", bufs=4) as wdp,
            tc.tile_pool(name="hp", bufs=2) as hp,
            tc.tile_pool(name="ysb", bufs=2) as ysbp,
            tc.tile_pool(name="gu_ps", bufs=4, space="PSUM") as gups,
            tc.tile_pool(name="y_ps", bufs=2, space="PSUM") as yps,
            tc.tile_pool(name="ht_ps", bufs=2, space="PSUM") as htps,
        ):
            ident = constp.tile([128, 128], f32, tag="ident")
            make_identity(nc, ident)

            # x^T resident for all 4 experts: [128, ko, token]
            xt = xtp.tile([128, KH, TC], f32r, tag="xt")
            nc.scalar.dma_start(
                out=xt[:], in_=xT.rearrange("(ko p) t -> p ko t", p=128)
            )

            y_pair = None
            for e in range(E_PER_CORE):
                te = e * T  # this expert's token column offset in xt

                # ---- gate/up: 4 PSUM accumulation groups over 16 K-tiles
                g0 = gups.tile([T, FH], f32, tag="gu")
                g1 = gups.tile([T, FH], f32, tag="gu")
                u0 = gups.tile([T, FH], f32, tag="gu")
                u1 = gups.tile([T, FH], f32, tag="gu")
                for c in range(KH // 2):  # 2 K-tiles per weight chunk
                    wgt = wgp.tile([128, 2, F], f32r, tag="wg")
                    nc.sync.dma_start(
                        out=wgt[:],
                        in_=wg[e, 256 * c : 256 * (c + 1), :].rearrange(
                            "(ko p) f -> p ko f", p=128
                        ),
                    )
                    wut = wup.tile([128, 2, F], f32r, tag="wu")
                    nc.sync.dma_start(
                        out=wut[:],
                        in_=wu[e, 256 * c : 256 * (c + 1), :].rearrange(
                            "(ko p) f -> p ko f", p=128
                        ),
                    )
                    for kk in range(2):
                        k = 2 * c + kk
                        st = k == 0
                        sp = k == KH - 1
                        lhs = xt[:, k, te : te + T]
                        nc.tensor.matmul(
                            g0[:], lhs, wgt[:, kk, 0:FH], start=st, stop=sp
                        )
                        nc.tensor.matmul(
                            g1[:], lhs, wgt[:, kk, FH:F], start=st, stop=sp
                        )
                        nc.tensor.matmul(
                            u0[:], lhs, wut[:, kk, 0:FH], start=st, stop=sp
                        )
                        nc.tensor.matmul(
                            u1[:], lhs, wut[:, kk, FH:F], start=st, stop=sp
                        )

                # ---- h = silu(g) * u
                h_silu = hp.tile([T, F], f32, tag="hsilu")
                nc.scalar.activation(
                    h_silu[:, 0:FH], g0[:], mybir.ActivationFunctionType.Silu
                )
                nc.scalar.activation(
                    h_silu[:, FH:F], g1[:], mybir.ActivationFunctionType.Silu
                )
                h = hp.tile([T, F], f32, tag="h")
                nc.vector.tensor_mul(h[:, 0:FH], h_silu[:, 0:FH], u0[:])
                nc.vector.tensor_mul(h[:, FH:F], h_silu[:, FH:F], u1[:])

                # ---- h^T via TensorE transposes into one PSUM bank
                ht_ps = htps.tile([128, KF, T], f32, tag="ht")
                for c in range(KF):
                    nc.tensor.transpose(
                        ht_ps[:, c, :], h[:, 128 * c : 128 * (c + 1)], ident[:T, :T]
                    )
                hT = hp.tile([128, KF, T], f32r, tag="hT")
                nc.vector.tensor_copy(out=hT[:], in_=ht_ps[:])

                # ---- down: y chunks of [64, 512] over 6 K-tiles
                if e % 2 == 0:
                    y_pair = ysbp.tile([128, H], f32, tag="ypair")
                prow = (e % 2) * T
                for nh in range(NHC):
                    wdt = wdp.tile([128, KF, NH], f32r, tag="wd")
                    nc.sync.dma_start(
                        out=wdt[:],
                        in_=wd[e, :, NH * nh : NH * (nh + 1)].rearrange(
                            "(ko p) hh -> p ko hh", p=128
                        ),
                    )
                    y_nh = yps.tile([T, NH], f32, tag="y")
                    for k in range(KF):
                        nc.tensor.matmul(
                            y_nh[:],
                            hT[:, k, :],
                            wdt[:, k, :],
                            start=(k == 0),
                            stop=(k == KF - 1),
                        )
                    # alternate PSUM->SBUF copies between ScalarE and VectorE
                    ydst = y_pair[prow : prow + T, NH * nh : NH * (nh + 1)]
                    if nh % 2 == 0:
                        nc.scalar.copy(out=ydst, in_=y_nh[:])
                    else:
                        nc.vector.tensor_copy(out=ydst, in_=y_nh[:])

                if e % 2 == 1:
                    pr = (e // 2) * 2 * T
                    nc.scalar.dma_start(
                        out=out[pr : pr + 2 * T, :], in_=y_pair[:]
                    )

    nc.compile()
    return nc


def _ensure_axon_hooks_stub():
    # concourse.bass_utils imports antenv.axon_hooks when tracing is
    # requested (e.g. BASS_TRACE=1 in the environment); the container's
    # antenv stub lacks that module.  Register a benign fallback so a
    # stray trace request degrades to "no profile" instead of crashing.
    import sys
    import types

    try:
        import antenv.axon_hooks  # noqa: F401
    except ImportError:
        m = types.ModuleType("antenv.axon_hooks")
        m.get_axon_ntff_profile_hook = lambda: None
        m.set_axon_ntff_profile_hook = lambda h: None
        sys.modules["antenv.axon_hooks"] = m


def _run(in_maps, trace=False):
    _ensure_axon_hooks_stub()
    from concourse.bass_utils import run_bass_kernel_spmd

    nc = _build_nc()
    return run_bass_kernel_spmd(
        nc, in_maps, list(range(N_CORES)), trace=trace
    )


def _make_in_maps(expert_tokens, gate_proj, up_proj, down_proj):
    x = np.ascontiguousarray(np.asarray(expert_tokens, dtype=np.float32))
    wg = np.asarray(gate_proj, dtype=np.float32)
    wu = np.asarray(up_proj, dtype=np.float32)
    wd = np.asarray(down_proj, dtype=np.float32)
    in_maps = []
    for c in range(N_CORES):
        er = slice(E_PER_CORE * c, E_PER_CORE * (c + 1))
        tr = slice(TC * c, TC * (c + 1))
        in_maps.append(
            {
                "xT": np.ascontiguousarray(x[tr].T),
                "wg": np.ascontiguousarray(wg[er]),
                "wu": np.ascontiguousarray(wu[er]),
                "wd": np.ascontiguousarray(wd[er]),
            }
        )
    return in_maps


def kernel(expert_tokens, expert_tokens_count, gate_proj, up_proj, down_proj):
    in_maps = _make_in_maps(expert_tokens, gate_proj, up_proj, down_proj)
    res = _run(in_maps, trace=False)
    y = np.concatenate(
        [res.results[c]["out"] for c in range(N_CORES)], axis=0
    )
    return np.asarray(y, dtype=np.float32)
